# revision 7
# baseline (speedup 1.0000x reference)
"""KMeans VQ-codebook kernel for Trainium2 (8 NeuronCores, data-parallel).

Computes out[n,k] = D[n,k] * onehot(argmin_k D[n,:]) where
D[n,k] = ||X[n] - V[k]||_2, for X [500000,128] f32, V [256,128] f32.

Per core (62500 rows):
  PE:   transpose X tile -> XT; fp32 matmul XT^T @ (-2V)^T -> M
  DVE:  tensor_tensor_reduce: Msb = M + v_sq, m = rowmin (one fused pass);
        u = m + x_sq; out = (Msb == m) * s  (fused is_equal+mult)
  ACT:  XT PSUM->SBUF copy; x_sq = rowsum(X^2) (Square w/ accum);
        s = sqrt(u)
The walrus build here accepts only ONE sync-wait per instruction, so all
HWDGE DMA completions are mapped onto a single semaphore lane and each
SBUF tile has exactly one reader engine (X is loaded twice: once for the
PE transpose, once for the ACT row-norm).
"""

import os
import sys

import numpy as np

sys.path.insert(0, "/opt/trn_rl_repo")

N = 500000
D = 128
K = 256
N_CORES = 8
NPC = N // N_CORES  # 62500 rows per core
P = 128

_nc_cache = {}


def _build(npc: int):
    from contextlib import ExitStack

    import concourse.bass as bass
    import concourse.tile as tile
    import concourse.tile_sem_assignment as tsa
    from concourse import mybir

    # This walrus rejects >1 sync-wait per instruction. All HWDGE DMAs on one
    # bookkeeping sem lane => any multi-DMA dependency folds to a single wait.
    tsa.NUM_HWDGE_SEMS = 1

    f32 = mybir.dt.float32
    Alu = mybir.AluOpType
    Act = mybir.ActivationFunctionType

    nc = bass.Bass(trn_type="TRN2")
    x_d = nc.dram_tensor("x", [npc, D], f32, kind="ExternalInput")
    wt_d = nc.dram_tensor("wt", [D, K], f32, kind="ExternalInput")
    vsq_d = nc.dram_tensor("vsq", [P, K], f32, kind="ExternalInput")
    id_d = nc.dram_tensor("ident", [P, P], f32, kind="ExternalInput")
    out_d = nc.dram_tensor("out", [npc, K], f32, kind="ExternalOutput")

    n_tiles = (npc + P - 1) // P

    def _split_multiwait():
        # This walrus build accepts at most ONE sync-wait per instruction.
        # Move all-but-the-last wait of any multi-wait instruction onto
        # freshly inserted single-wait Drain instructions just before it
        # (same engine, so ordering semantics are identical).
        cnt = 0
        for fn in nc.m.functions:
            for bb in fn.blocks:
                insts = list(bb.instructions)
                out = []
                changed = False
                for ins in insts:
                    si = getattr(ins, "sync_info", None)
                    waits = list(si.on_wait) if (si and si.on_wait) else []
                    if len(waits) > 1:
                        changed = True
                        for w in waits[:-1]:
                            cnt += 1
                            dr = mybir.InstDrain(
                                name=f"antw-{cnt}", ins=[], outs=[]
                            )
                            dr.engine = ins.engine
                            dr.sync_info = mybir.SyncInfo(
                                on_wait=[w], on_update=[]
                            )
                            out.append(dr)
                        ins.sync_info = mybir.SyncInfo(
                            on_wait=[waits[-1]], on_update=list(si.on_update)
                        )
                    out.append(ins)
                if changed:
                    bb.instructions = out
        return cnt

    with tile.TileContext(nc) as tc, ExitStack() as ctx:
        singles = ctx.enter_context(tc.tile_pool(name="singles", bufs=1))
        wt_sb = singles.tile([D, K], f32)
        nc.sync.dma_start(out=wt_sb, in_=wt_d[:, :])
        vsq_sb = singles.tile([P, K], f32)
        nc.sync.dma_start(out=vsq_sb, in_=vsq_d[:, :])
        id_sb = singles.tile([P, P], f32)
        nc.sync.dma_start(out=id_sb, in_=id_d[:, :])

        xpool = ctx.enter_context(tc.tile_pool(name="xin", bufs=4))
        xqpool = ctx.enter_context(tc.tile_pool(name="xq", bufs=4))
        xtps = ctx.enter_context(tc.tile_pool(name="xtps", bufs=2, space="PSUM"))
        xtsb = ctx.enter_context(tc.tile_pool(name="xtsb", bufs=3))
        mps = ctx.enter_context(tc.tile_pool(name="mps", bufs=3, space="PSUM"))
        msb = ctx.enter_context(tc.tile_pool(name="msb", bufs=3))
        outp = ctx.enter_context(tc.tile_pool(name="outp", bufs=4))
        smalls = ctx.enter_context(tc.tile_pool(name="smalls", bufs=4))
        junkp = ctx.enter_context(tc.tile_pool(name="junk", bufs=2))

        for t in range(n_tiles):
            row0 = min(t * P, npc - P)
            # two loads: x_t is read only by PE (transpose), x_q only by ACT
            x_t = xpool.tile([P, D], f32)
            nc.sync.dma_start(out=x_t, in_=x_d[row0 : row0 + P, :])
            x_q = xqpool.tile([P, D], f32)
            nc.sync.dma_start(out=x_q, in_=x_d[row0 : row0 + P, :])

            xt_ps = xtps.tile([P, P], f32)
            nc.tensor.transpose(xt_ps, x_t, id_sb)
            xt = xtsb.tile([P, P], f32)
            nc.scalar.copy(xt, xt_ps)

            xsq = smalls.tile([P, 1], f32, tag="xsq")
            junk = junkp.tile([P, D], f32)
            nc.scalar.activation(junk, x_q, Act.Square, accum_out=xsq)

            m_ps = mps.tile([P, K], f32)
            nc.tensor.matmul(m_ps, lhsT=xt, rhs=wt_sb, start=True, stop=True)

            m_s = msb.tile([P, K], f32, tag="msb")
            mrow = smalls.tile([P, 1], f32, tag="mrow")
            nc.vector.tensor_tensor(
                out=m_s, in0=m_ps, in1=vsq_sb, op=Alu.add
            )
            nc.vector.tensor_reduce(
                out=mrow, in_=m_s, axis=mybir.AxisListType.X, op=Alu.min
            )

            u = smalls.tile([P, 1], f32, tag="u")
            nc.vector.tensor_add(u, mrow, xsq)
            s_val = smalls.tile([P, 1], f32, tag="sval")
            nc.scalar.activation(s_val, u, Act.Sqrt)

            o_t = outp.tile([P, K], f32)
            nc.vector.tensor_scalar(
                out=o_t,
                in0=m_s,
                scalar1=mrow,
                scalar2=s_val,
                op0=Alu.is_equal,
                op1=Alu.mult,
            )
            nc.sync.dma_start(out=out_d[row0 : row0 + P, :], in_=o_t)

    _split_multiwait()
    return nc


def _host_prep(V: np.ndarray):
    V = np.asarray(V, dtype=np.float32)
    wt = np.ascontiguousarray((-2.0 * V).T)  # [D, K]
    vsq = np.sum(V * V, axis=1, dtype=np.float32)  # [K]
    vsq_b = np.ascontiguousarray(np.broadcast_to(vsq[None, :], (P, K)))
    ident = np.eye(P, dtype=np.float32)
    return wt, vsq_b, ident


def kernel(X: np.ndarray, V: np.ndarray) -> np.ndarray:
    from concourse.bass_utils import run_bass_kernel_spmd

    X = np.ascontiguousarray(np.asarray(X, dtype=np.float32))
    wt, vsq_b, ident = _host_prep(V)

    if "full" not in _nc_cache:
        _nc_cache["full"] = _build(NPC)
    nc = _nc_cache["full"]

    in_maps = [
        {
            "x": np.ascontiguousarray(X[c * NPC : (c + 1) * NPC]),
            "wt": wt,
            "vsq": vsq_b,
            "ident": ident,
        }
        for c in range(N_CORES)
    ]

    trace = bool(int(os.environ.get("KMEANS_TRACE", "0")))
    res = run_bass_kernel_spmd(
        nc, in_maps, core_ids=list(range(N_CORES)), trace=trace
    )
    if trace and res.exec_time_ns is not None:
        kernel.last_exec_time_ns = res.exec_time_ns
        kernel.last_mean_exec_time_ns = res.mean_exec_time_ns
        kernel.last_trace = res.instructions_and_trace
    out = np.concatenate([r["out"] for r in res.results], axis=0)
    return out


kernel.last_exec_time_ns = None
kernel.last_mean_exec_time_ns = None
kernel.last_trace = None
